# revision 9
# baseline (speedup 1.0000x reference)
"""3-layer GCN (gcn_norm message passing) on 8 Trainium2 NeuronCores.

Architecture (v5):
  - Nodes row-sharded across 8 cores (12500 real + 44 pad rows each); per
    layer each core computes h_mm = relu(h_prev) @ W for its shard, scaled by
    dis[src] (norm factorization: norm = dis[dest]*dis[src]), AllGathers the
    bf16 table, then aggregates messages for the destinations it owns.
  - Messages sorted by (dest-group of 4 blocks, source-quarter, dest-block).
    Per (block, quarter) runs are padded to 16-slot units so run boundaries
    are identical on all cores; one dma_gather per (group, quarter) on 4
    parallel SWDGE queues (int16 indices address the bf16 table through 4
    row-windows of 25088 rows).
  - Segment-sum on the TensorEngine: chunks ordered quarter-major; each
    128-message chunk issues ONE matmul covering its contiguous dest-block
    span (up to 4 blocks wide) into the group's [128, 512] PSUM tile. The
    group's first chunk streams full width with start=True (zero-init), the
    last with stop=True; middle chunks accumulate over their span only.
  - One-hots are bf16 iota==meta compares (DVE 2-byte mode), built 8
    chunk-columns at a time; multi-block chunks occupy adjacent columns with
    meta values offset by -128 per block.
  - Group epilogue: out = psum * dis[dest] (DVE wide) + bias (ACT Identity),
    relu (ACT wide), next-layer matmuls (PE), hm = psum2 * dis[own] (ACT).

All data-dependent structure is baked at trace time; the NEFF is compiled
per call and cached in-process.
"""

import os
import sys

sys.path.insert(0, "/opt/trn_rl_repo")

import numpy as np
import ml_dtypes

from concourse import bacc, bass, mybir
from concourse import tile
from concourse import bass_utils

F32 = mybir.dt.float32
BF16 = mybir.dt.bfloat16
I16 = mybir.dt.int16

N_CORES = 8
NQ = 4       # source windows (int16 index range / table rows)
G = 4        # dest blocks per gather group
WOH = 8      # one-hot columns per wide DVE op / tile
RUN = 16     # run alignment granularity (slots)
PAD_SEG = 10000.0


def _schedule(caps, ngrp, nblk):
    """Static layout shared by prep and builder.

    caps: [nblk][NQ] per-(block, quarter) run capacity in RUN-slot units.
    Chunks are quarter-major within a group; each chunk gets one matmul
    covering its contiguous local-block span [j0, j1]. The group's first and
    last chunks are widened to the full group width for start/stop flags.
    """
    call_cols = np.zeros(ngrp * NQ + 1, dtype=np.int64)   # gidx col base
    chunk_base = np.zeros(ngrp * NQ + 1, dtype=np.int64)  # chunk id base
    call_nidx = []
    run_slot = {}   # (b, q) -> slot offset of run inside its call
    groups = []
    tile_col_base = 0  # global one-hot column counter (128-wide cols)
    for g in range(ngrp):
        blocks = list(range(g * G, min((g + 1) * G, nblk)))
        nj = len(blocks)
        chunk_spans = []
        for q in range(NQ):
            off = 0
            spans = []
            for j, b in enumerate(blocks):
                run_slot[(b, q)] = off
                spans.append((j, off, off + caps[b][q] * RUN))
                off += caps[b][q] * RUN
            nidx = ((off + 127) // 128) * 128
            call_nidx.append(nidx)
            call_cols[g * NQ + q + 1] = call_cols[g * NQ + q] + nidx // 16
            chunk_base[g * NQ + q + 1] = chunk_base[g * NQ + q] + nidx // 128
            for c in range(nidx // 128):
                lo, hi = c * 128, (c + 1) * 128
                js = [j for j, s0, s1 in spans if s0 < hi and s1 > lo]
                if not js:
                    js = [nj - 1]   # pure-pad chunk beyond the last run
                chunk_spans.append((q, c, min(js), max(js)))
        n = len(chunk_spans)
        mms = []
        t, o = 0, 0
        for k, (q, c, j0, j1) in enumerate(chunk_spans):
            if k == 0 or k == n - 1:
                j0, j1 = 0, nj - 1   # full width for start/stop bracketing
            w = j1 - j0 + 1
            if o + w > WOH:
                t += 1
                o = 0
            mms.append({
                "q": q, "c": c, "j0": j0, "j1": j1, "tile": t, "off": o,
                "start": k == 0, "stop": k == n - 1,
            })
            o += w
        ntiles = t + 1
        groups.append({
            "mms": mms, "ntiles": ntiles, "nj": nj,
            "col_base": tile_col_base,
        })
        tile_col_base += ntiles * WOH
    return {
        "call_cols": call_cols, "chunk_base": chunk_base,
        "call_nidx": call_nidx, "run_slot": run_slot,
        "groups": groups,
        "gidx_cols": int(call_cols[-1]),
        "total_chunks": int(chunk_base[-1]),
        "total_cols": tile_col_base,
    }


# ----------------------------------------------------------------------------
# Host-side preparation
# ----------------------------------------------------------------------------

def _prep_inputs(x, edge_index, W0, b0, W1, b1, W2, b2, s_real):
    n = x.shape[0]
    assert n % N_CORES == 0 and s_real == n // N_CORES
    nblk = (s_real + 127) // 128
    s_pad = nblk * 128
    total = N_CORES * s_pad
    ngrp = (nblk + G - 1) // G
    assert total % NQ == 0
    wq = total // NQ
    assert wq <= 32767, f"window {wq} exceeds int16 range"

    d = np.asarray(edge_index[0], dtype=np.int64)
    s = np.asarray(edge_index[1], dtype=np.int64)

    deg = np.bincount(s, minlength=n).astype(np.float64) + 1.0
    dis = (1.0 / np.sqrt(deg)).astype(np.float32)

    dests = np.concatenate([d, np.arange(n, dtype=np.int64)])
    srcs = np.concatenate([s, np.arange(n, dtype=np.int64)])

    core = dests // s_real
    dloc = dests - core * s_real
    blk = dloc >> 7
    grp = blk // G
    jj = blk - grp * G
    sg = (srcs // s_real) * s_pad + (srcs % s_real)
    q = sg // wq
    widx = (sg - q * wq).astype(np.int64)

    # per-(core, block, quarter) counts -> shared run capacities (RUN units)
    key = (core * nblk + blk) * NQ + q
    counts = np.bincount(key, minlength=N_CORES * nblk * NQ).reshape(
        N_CORES, nblk, NQ
    )
    caps = np.maximum(
        (counts.max(axis=0) + RUN - 1) // RUN, 1
    )  # [nblk, NQ]

    lay = _schedule(caps.tolist(), ngrp, nblk)

    # rank within (core, b, q)
    order = np.argsort(key, kind="stable")
    inv = np.empty_like(order)
    inv[order] = np.arange(order.size)
    starts = np.zeros(N_CORES * nblk * NQ + 1, dtype=np.int64)
    np.cumsum(counts.reshape(-1), out=starts[1:])
    rank = inv - starts[key]

    run_slot_arr = np.zeros((nblk, NQ), dtype=np.int64)
    for (b, qq), v in lay["run_slot"].items():
        run_slot_arr[b, qq] = v

    slot = run_slot_arr[blk, q] + rank               # slot within call
    call_id = grp * NQ + q
    gcol = lay["call_cols"][call_id] + (slot >> 4)
    grow = slot & 15
    gchunk = lay["chunk_base"][call_id] + (slot >> 7)
    part = slot & 127

    gidx16 = np.zeros((N_CORES, 16, lay["gidx_cols"]), dtype=np.int16)
    gidx16[core, grow, gcol] = widx.astype(np.int16)
    gidx = np.broadcast_to(
        gidx16[:, None, :, :], (N_CORES, 8, 16, lay["gidx_cols"])
    ).reshape(N_CORES, 128, lay["gidx_cols"]).copy()

    # chunk -> (one-hot column 0, local j of column 0)
    chunk_col0 = np.full(lay["total_chunks"], -1, dtype=np.int64)
    chunk_j0 = np.zeros(lay["total_chunks"], dtype=np.int64)
    chunk_j1 = np.zeros(lay["total_chunks"], dtype=np.int64)
    for g in range(ngrp):
        grp_lay = lay["groups"][g]
        cb = lay["chunk_base"]
        for mm in grp_lay["mms"]:
            gc = cb[g * NQ + mm["q"]] + mm["c"]
            chunk_col0[gc] = grp_lay["col_base"] + mm["tile"] * WOH + mm["off"]
            chunk_j0[gc] = mm["j0"]
            chunk_j1[gc] = mm["j1"]
    assert (chunk_col0 >= 0).all()

    assert ((jj >= chunk_j0[gchunk]) & (jj <= chunk_j1[gchunk])).all()
    col = chunk_col0[gchunk] + (jj - chunk_j0[gchunk])
    meta_f = np.full((N_CORES, 128, lay["total_cols"]), PAD_SEG, np.float32)
    meta_f[core, part, col] = (dloc - blk * 128).astype(np.float32)
    meta = meta_f.astype(ml_dtypes.bfloat16)

    # dense inputs
    x = np.asarray(x, dtype=np.float32)
    x_t = np.zeros((N_CORES, 128, s_pad), dtype=np.float32)
    dison = np.zeros((N_CORES, 128, nblk), dtype=np.float32)
    disd = np.zeros((N_CORES, 128, s_pad), dtype=np.float32)
    for r in range(N_CORES):
        x_t[r, :, :s_real] = x[r * s_real : (r + 1) * s_real].T
        dv = np.zeros(s_pad, dtype=np.float32)
        dv[:s_real] = dis[r * s_real : (r + 1) * s_real]
        dison[r] = dv.reshape(nblk, 128).T
        disd[r] = dv[None, :]

    wdata = np.zeros((128, 3 * 128 + 3), dtype=np.float32)
    wdata[:, 0:128] = np.asarray(W0, dtype=np.float32)
    wdata[:, 128:256] = np.asarray(W1, dtype=np.float32)
    wdata[:, 256:384] = np.asarray(W2, dtype=np.float32)
    wdata[:, 384] = np.asarray(b0, dtype=np.float32)
    wdata[:, 385] = np.asarray(b1, dtype=np.float32)
    wdata[:, 386] = np.asarray(b2, dtype=np.float32)
    iotar = np.tile(
        np.arange(128, dtype=np.float32), WOH
    )[None, :].repeat(128, axis=0).astype(ml_dtypes.bfloat16)

    in_maps = [
        {
            "x_t": x_t[r], "meta": meta[r], "gidx": gidx[r],
            "wdata": wdata, "iotar": iotar, "dison": dison[r],
            "disd": disd[r],
        }
        for r in range(N_CORES)
    ]
    sched = {
        "nblk": nblk, "s_pad": s_pad, "s_real": s_real, "ngrp": ngrp,
        "caps": caps.tolist(),
    }
    return in_maps, sched


# ----------------------------------------------------------------------------
# Device kernel builder
# ----------------------------------------------------------------------------

def build_kernel(sched, n_cores=N_CORES):
    from contextlib import ExitStack

    nblk, s_pad, ngrp = sched["nblk"], sched["s_pad"], sched["ngrp"]
    caps = sched["caps"]
    lay = _schedule(caps, ngrp, nblk)
    total = n_cores * s_pad
    wq = total // NQ

    nc = bacc.Bacc(
        "TRN2", target_bir_lowering=False, debug=False, num_devices=n_cores,
        num_swdge_queues=NQ,
        # Per-queue SWDGE descriptor-ring capacity is scratch_size//16 descs;
        # the default 16 KiB (1024 descs) is smaller than one ~2500-desc
        # gather call, so the Pool engine blocks inside every dma_gather at
        # SDMA drain rate. 64 KiB holds a full call plus most of the next.
        dynamic_dma_scratch_size=65536,
    )
    x_t = nc.dram_tensor("x_t", [128, s_pad], F32, kind="ExternalInput")
    meta = nc.dram_tensor(
        "meta", [128, lay["total_cols"]], BF16, kind="ExternalInput"
    )
    gidx = nc.dram_tensor("gidx", [128, lay["gidx_cols"]], I16, kind="ExternalInput")
    wdata = nc.dram_tensor("wdata", [128, 3 * 128 + 3], F32, kind="ExternalInput")
    iotar = nc.dram_tensor("iotar", [128, WOH * 128], BF16, kind="ExternalInput")
    dison = nc.dram_tensor("dison", [128, nblk], F32, kind="ExternalInput")
    disd = nc.dram_tensor("disd", [128, s_pad], F32, kind="ExternalInput")
    h_out = nc.dram_tensor("h_out", [128, 3 * s_pad], F32, kind="ExternalOutput")

    rg = [list(range(n_cores))]
    ID = mybir.ActivationFunctionType

    with tile.TileContext(nc) as tc, ExitStack() as ctx:
        const = ctx.enter_context(tc.tile_pool(name="const", bufs=1))
        dram = ctx.enter_context(tc.tile_pool(name="dram", bufs=1, space="DRAM"))
        xw = ctx.enter_context(tc.tile_pool(name="xw", bufs=4))
        hmm = ctx.enter_context(tc.tile_pool(name="hmm", bufs=6))
        gath = ctx.enter_context(tc.tile_pool(name="gath", bufs=4 * NQ))
        idxp = ctx.enter_context(tc.tile_pool(name="idxp", bufs=4 * NQ))
        metat = ctx.enter_context(tc.tile_pool(name="metat", bufs=6))
        ohp = ctx.enter_context(tc.tile_pool(name="ohp", bufs=20))
        outsb = ctx.enter_context(tc.tile_pool(name="outsb", bufs=3))
        ddp = ctx.enter_context(tc.tile_pool(name="ddp", bufs=2))
        rsb = ctx.enter_context(tc.tile_pool(name="rsb", bufs=3))
        agg_ps = ctx.enter_context(tc.tile_pool(name="agg_ps", bufs=3, space="PSUM"))
        mm_ps = ctx.enter_context(tc.tile_pool(name="mm_ps", bufs=2, space="PSUM"))
        mma_ps = ctx.enter_context(tc.tile_pool(name="mma_ps", bufs=2, space="PSUM"))

        ag_in = dram.tile([s_pad, 128], BF16)
        ag_outs = [
            dram.tile([total, 128], BF16, addr_space="Shared", name=f"ag_out_l{i}")
            for i in range(3)
        ]

        w_sb = const.tile([128, 3 * 128 + 3], F32)
        nc.sync.dma_start(out=w_sb[:], in_=wdata[:])
        w_bf = const.tile([128, 3 * 128], BF16)
        nc.vector.tensor_copy(w_bf[:], w_sb[:, 0 : 3 * 128])
        iota_sb = const.tile([128, WOH * 128], BF16)
        nc.sync.dma_start(out=iota_sb[:], in_=iotar[:])
        dison_sb = const.tile([128, nblk], F32)
        nc.sync.dma_start(out=dison_sb[:], in_=dison[:])

        def bias(L):
            return w_sb[:, 384 + L : 385 + L]

        # ---- Phase A: table0 = (x @ W0) * dis -> ag_in ----
        for b in range(nblk):
            xt = xw.tile([128, 128], F32)
            nc.sync.dma_start(out=xt[:], in_=x_t[:, b * 128 : (b + 1) * 128])
            ps = mma_ps.tile([128, 128], F32, name="psA", tag="psA")
            nc.tensor.matmul(
                ps[:], lhsT=xt[:], rhs=w_sb[:, 0:128], start=True, stop=True
            )
            hm = hmm.tile([128, 128], BF16, name="hmA", tag="hm")
            nc.scalar.activation(
                hm[:], ps[:], ID.Copy, scale=dison_sb[:, b : b + 1]
            )
            nc.scalar.dma_start(out=ag_in[b * 128 : (b + 1) * 128, :], in_=hm[:])

        # ---- 3 layers ----
        def flush_pending(pending, L):
            """Next-layer matmuls + table writes for a finished group.

            Emitted one group late so the PE (in-order) doesn't stall the
            next group's aggregation matmuls behind the epilogue ACT chain.
            """
            if pending is None:
                return
            r, blocks, nj, w = pending
            ps2 = mm_ps.tile([128, G * 128], F32, name="ps2", tag="ps2")
            for j in range(nj):
                nc.tensor.matmul(
                    ps2[:, j * 128 : (j + 1) * 128],
                    lhsT=r[:, j * 128 : (j + 1) * 128],
                    rhs=w_bf[:, (L + 1) * 128 : (L + 2) * 128],
                    start=True,
                    stop=True,
                    skip_group_check=True,
                )
            for j in range(nj):
                b = blocks[j]
                hm = hmm.tile([128, 128], BF16, name="hm", tag="hm")
                nc.scalar.activation(
                    hm[:], ps2[:, j * 128 : (j + 1) * 128], ID.Copy,
                    scale=dison_sb[:, b : b + 1],
                )
                nc.scalar.dma_start(
                    out=ag_in[b * 128 : (b + 1) * 128, :], in_=hm[:]
                )

        for L in range(3):
            ag_out = ag_outs[L]
            nc.gpsimd.collective_compute(
                "AllGather",
                mybir.AluOpType.bypass,
                replica_groups=rg,
                ins=[ag_in[:].opt()],
                outs=[ag_out[:].opt()],
            )
            pending = None
            for g in range(ngrp):
                grp = lay["groups"][g]
                nj = grp["nj"]
                blocks = list(range(g * G, g * G + nj))
                gts = []
                for q in range(NQ):
                    nidx = lay["call_nidx"][g * NQ + q]
                    c0 = lay["call_cols"][g * NQ + q]
                    it = idxp.tile([128, nidx // 16], I16, name="it", tag="it")
                    nc.sync.dma_start(
                        out=it[:], in_=gidx[:, c0 : c0 + nidx // 16]
                    )
                    gt = gath.tile([128, nidx], BF16, name="gt", tag="gt")
                    nc.gpsimd.dma_gather(
                        gt[:].rearrange("p (c f) -> p c f", f=128),
                        ag_out[q * wq : (q + 1) * wq, :],
                        it[:],
                        num_idxs=nidx,
                        num_idxs_reg=nidx,
                        elem_size=128,
                        elem_step=128,
                        single_packet=(nidx <= 1024),
                        queue_num=q,
                    )
                    gts.append(gt)

                ntiles = grp["ntiles"]
                cb0 = grp["col_base"]
                mt = metat.tile([128, ntiles * WOH], BF16)
                nc.sync.dma_start(
                    out=mt[:], in_=meta[:, cb0 : cb0 + ntiles * WOH]
                )
                ohs = {}
                for t in range(ntiles):
                    oh = ohp.tile([128, WOH * 128], BF16, name="oh", tag="oh")
                    nc.vector.tensor_tensor(
                        oh[:].rearrange("p (c f) -> p c f", f=128),
                        iota_sb[:].rearrange("p (c f) -> p c f", f=128),
                        mt[:, t * WOH : (t + 1) * WOH].to_broadcast(
                            [128, WOH, 128]
                        ),
                        mybir.AluOpType.is_equal,
                    )
                    ohs[t] = oh

                ps = agg_ps.tile([128, G * 128], F32, name="aggps", tag="aggps")
                for mm in grp["mms"]:
                    w = mm["j1"] - mm["j0"] + 1
                    o = mm["off"]
                    nc.tensor.matmul(
                        ps[:, mm["j0"] * 128 : (mm["j1"] + 1) * 128],
                        lhsT=gts[mm["q"]][:, mm["c"] * 128 : (mm["c"] + 1) * 128],
                        rhs=ohs[mm["tile"]][:, o * 128 : (o + w) * 128],
                        start=mm["start"],
                        stop=mm["stop"],
                        skip_group_check=True,
                    )
                # next-layer work for the previous group, after this group's
                # agg matmuls so the PE never waits on the epilogue chain
                flush_pending(pending, L)
                pending = None

                # ---- group epilogue ----
                w = nj * 128
                gb = g * G * 128
                dd = ddp.tile([128, G * 128], F32, name="dd", tag="dd")
                nc.scalar.dma_start(out=dd[:, :w], in_=disd[:, gb : gb + w])
                ob = outsb.tile([128, G * 128], F32, name="ob", tag="ob")
                nc.vector.tensor_tensor(
                    ob[:, :w], ps[:, :w], dd[:, :w], mybir.AluOpType.mult
                )
                nc.scalar.activation(ob[:, :w], ob[:, :w], ID.Identity, bias=bias(L))
                nc.sync.dma_start(
                    out=h_out[:, L * s_pad + gb : L * s_pad + gb + w],
                    in_=ob[:, :w],
                )
                if L < 2:
                    r = rsb.tile([128, G * 128], BF16, name="r", tag="r")
                    nc.scalar.activation(r[:, :w], ob[:, :w], ID.Relu)
                    pending = (r, blocks, nj, w)
            flush_pending(pending, L)

    nc.compile()
    return nc


_BUILD_CACHE = {}


def _get_kernel(sched):
    key = (
        sched["nblk"], sched["s_pad"],
        tuple(tuple(c) for c in sched["caps"]),
    )
    if key not in _BUILD_CACHE:
        _BUILD_CACHE[key] = build_kernel(sched)
    return _BUILD_CACHE[key]


# ----------------------------------------------------------------------------
# Entry point
# ----------------------------------------------------------------------------

def _run(x, edge_index, W0, b0, W1, b1, W2, b2, trace=False):
    n = int(np.asarray(x).shape[0])
    s_real = n // N_CORES
    in_maps, sched = _prep_inputs(
        x, edge_index, W0, b0, W1, b1, W2, b2, s_real
    )
    s_pad = sched["s_pad"]
    nc = _get_kernel(sched)
    res = bass_utils.run_bass_kernel_spmd(
        nc, in_maps, core_ids=list(range(N_CORES)), trace=trace
    )
    outs = []
    for L in range(3):
        h = np.concatenate(
            [
                res.results[r]["h_out"][:, L * s_pad : L * s_pad + s_real]
                for r in range(N_CORES)
            ],
            axis=1,
        ).T
        outs.append(h)
    full = np.stack(outs, axis=1).astype(np.float32)
    return full, res


def kernel(**inputs):
    trace = os.environ.get("TRN_KERNEL_TRACE", "") == "1"
    out, res = _run(
        np.asarray(inputs["x"]),
        np.asarray(inputs["edge_index"]),
        np.asarray(inputs["W0"]),
        np.asarray(inputs["b0"]),
        np.asarray(inputs["W1"]),
        np.asarray(inputs["b1"]),
        np.asarray(inputs["W2"]),
        np.asarray(inputs["b2"]),
        trace=trace,
    )
    if trace and res.exec_time_ns is not None:
        print(f"HW exec time: {res.exec_time_ns} ns")
        if res.instructions_and_trace:
            print(f"trace: {res.instructions_and_trace[1]}")
    return out
